# revision 38
# baseline (speedup 1.0000x reference)
"""RNN-T decoder (embedding + 2-layer LSTM + joint network) on 8 Trainium2 cores.

Strategy (v2, fp8 DoubleRow):
  - LSTM (B=4, U=64, D=1024) runs redundantly on all 8 cores. The recurrent
    matmul streams W_hh (pre-scaled x32, fp8e4m3, DoubleRow-interleaved) through
    the PE at 0.5 cyc/row with the tiny h^T chunk (fp8) stationary. The batched
    x-projections are fp8 DoubleRow too, injected into the gate PSUM via a
    constant selector matrix. Activation scale 1/32 undoes the weight scaling.
  - Tokens are b-major (tok = 64*b + u) so every hT/gxT consumer is a clean
    contiguous or 2-level-strided AP.
  - Joint network (dominant FLOPs) is sharded over T: core c computes
    out[:, c*16:(c+1)*16, :, :]. Stays f32r for precision; dec_p uses fp8
    wdec (x16) with the scale folded into the z-tanh activation.

kernel(**inputs) takes the full unsharded inputs (as in reference.setup_inputs)
and returns the full (B, T, U, ODIM) float32 output.
"""
import sys
import numpy as np
import ml_dtypes

sys.path.insert(0, "/opt/trn_rl_repo")

import concourse.bass as bass
import concourse.bacc as bacc
import concourse.mybir as mybir
import concourse.tile as tile
from contextlib import ExitStack

F32 = mybir.dt.float32
F32R = mybir.dt.float32r
BF16 = mybir.dt.bfloat16
F8 = mybir.dt.float8e4
I32 = mybir.dt.int32
AF = mybir.ActivationFunctionType
OP = mybir.AluOpType
DRM = mybir.MatmulPerfMode.DoubleRow
E4NP = ml_dtypes.float8_e4m3

B, T, U = 4, 128, 64
NCORES = 8
TC = T // NCORES           # 16 T-columns per core
E, D, G = 512, 1024, 4096  # embed, dunits, 4*dunits
J, O = 512, 2048           # joint dim, odim
UB = U * B                 # 256 tokens, b-major: tok = 64*b + u
BT = B * TC                # 64 encoder rows per core
KP = D // 256              # 4 contraction pairs of the hidden dim
SW = 32.0                  # gate pre-activation scale (weights x32)
SDEC = 16.0                # wdec scale

_CACHE = {}


def _mm_r(nc, out, lhsT, rhs, **kw):
    nc.tensor.matmul(out, lhsT=lhsT.bitcast(F32R), rhs=rhs.bitcast(F32R), **kw)


def _mm8(nc, out, lhsT, rhs, start, stop):
    nc.tensor.matmul(out, lhsT=lhsT, rhs=rhs, start=start, stop=stop,
                     perf_mode=DRM)


def _emit_xproj8(nc, pools, w8, nchunks, rhs_of_c, biasT, gxT8s):
    """gxT8s[pc][:, :, gm&1, 0:4] = fp8( (W x)^T + bias ), 32 gate-chunks.

    w8: list of [128, 2, G] fp8 tiles (DR-interleaved W.T, pre-scaled x32)
    rhs_of_c: c -> [128, 2, 256] fp8 moving operand (x^T pair chunk, u-major)
    biasT: [128, 32] f32 tile; bias for gate-chunk gm is biasT[:, gm:gm+1]
    gxT8s: 16 tiles [128, 64, 2, 16] fp8 (aligned stationary layout)
    """
    pbig = pools["pbig"]
    for gm in range(32):
        ps = pbig.tile([128, 256], F32, tag="pbig", name="pbig")
        for c in range(nchunks):
            _mm8(nc, ps[:], w8[c][:, :, 128 * gm:128 * (gm + 1)], rhs_of_c(c),
                 start=(c == 0), stop=(c == nchunks - 1))
        # PSUM -> fp8 gxT8s with per-partition bias; DVE (ACT-table free)
        nc.vector.tensor_scalar(gxT8s[gm >> 1][:, :, gm & 1, 0:4], ps[:],
                                biasT[:, gm:gm + 1], None, OP.add)


def _emit_lstm_layer(nc, pools, ident, whh8, zT8, gxT8s, hT8s, c_sb, gates,
                     hbuf, steps):
    """One LSTM layer, `steps` sequential steps, fp8 DoubleRow h-matmuls.

    All activations use the Tanh table only (sigmoid via 0.5*tanh(x/2)+0.5,
    fixup on Pool/DVE) to avoid ACT table switching.

    whh8:  4 tiles [128, 2, G] fp8 = W_hh.T DR pairs (moving operand, x32)
    gxT8s: 16 tiles [128, 64, 2, 16] fp8 = (W_ih x + b)^T x32, aligned
           stationary layout for DR injects via the sel8 selector
    hT8s:  4 tiles [128, 64, 2, 16] fp8; [c][:, u, i, m] = h_u[m, 256c+128i+p]
           written at slot u after step u; step u reads slot u-1 (zT8 u=0).
    """
    pgates, ptr = pools["pgates"], pools["ptr"]
    sel8 = pools["sel8"]
    ident4 = ident[:4, :4]
    for u in range(steps):
        for gi in (2, 1, 0, 3):  # g, f, i, o
            gs = slice(1024 * gi, 1024 * (gi + 1))
            ps = pgates.tile([4, 1024], F32, tag="pg", name="pg")
            for pc4 in range(4):
                # start on the first write of each 2KB PSUM bank (zero region)
                nc.tensor.matmul(
                    ps[:, 256 * pc4:256 * (pc4 + 1)],
                    lhsT=gxT8s[4 * gi + pc4][:, u, :, 0:4], rhs=sel8[:],
                    start=(pc4 in (0, 2)), stop=False, perf_mode=DRM)
            for c in range(4):
                lt = zT8[:, :, 0:4] if u == 0 else hT8s[c][:, u - 1, :, 0:4]
                for hf in range(2):
                    nc.tensor.matmul(
                        ps[:, 512 * hf:512 * (hf + 1)], lhsT=lt,
                        rhs=whh8[c][:, :, 1024 * gi + 512 * hf:
                                    1024 * gi + 512 * (hf + 1)],
                        start=False, stop=(c == 3), perf_mode=DRM)
            fn = AF.Tanh if gi == 2 else AF.Sigmoid
            nc.scalar.activation(gates[:, gs], ps[:], fn, scale=1.0 / SW)
        # c' = sig(f)*c + sig(i)*tanh(g);  h = sig(o)*tanh(c')
        for hh in range(2):
            s = slice(512 * hh, 512 * (hh + 1))
            gi_ = gates[:, 0 * D:][:, s]
            gf = gates[:, 1 * D:][:, s]
            gg = gates[:, 2 * D:][:, s]
            go = gates[:, 3 * D:][:, s]
            ch = c_sb[:, s]
            nc.vector.tensor_tensor(gf, in0=gf, in1=ch, op=OP.mult)    # f*c
            nc.gpsimd.tensor_tensor(gi_, in0=gi_, in1=gg, op=OP.mult)  # i*g~
            nc.gpsimd.tensor_tensor(ch, in0=gf, in1=gi_, op=OP.add)    # c'
        nc.scalar.activation(gates[:, 2 * D:3 * D], c_sb[:], AF.Tanh)  # tanh(c')
        for hh in range(2):
            s = slice(512 * hh, 512 * (hh + 1))
            gg = gates[:, 2 * D:][:, s]
            go = gates[:, 3 * D:][:, s]
            nc.vector.tensor_tensor(hbuf[:, s], in0=go, in1=gg, op=OP.mult)
            for k2 in range(2 * hh, 2 * hh + 2):
                # two chunk-transposes share one PSUM tile -> single fp8 copy
                tp = ptr.tile([128, 8], F32, tag="tr", name="tr")
                for i in range(2):
                    k = 2 * k2 + i
                    nc.tensor.matmul(
                        tp[:, 4 * i:4 * (i + 1)],
                        lhsT=hbuf[:, 128 * k:128 * (k + 1)], rhs=ident4,
                        is_transpose=True, start=(i == 0), stop=(i == 1))
                nc.vector.tensor_copy(hT8s[k2][:, u, :, 0:4], tp[:])


def build_nc(steps=U):
    nc = bacc.Bacc("TRN2", target_bir_lowering=False, debug=False)

    hs = nc.dram_tensor("hs", [BT, E], F32, kind="ExternalInput")
    ys_idx = nc.dram_tensor("ys_idx", [UB], I32, kind="ExternalInput")
    embed8 = nc.dram_tensor("embed8", [O, E], F32, kind="ExternalInput")
    wih0_8d = nc.dram_tensor("wih0_8", [256, 2 * G], F8, kind="ExternalInput")
    whh0_8d = nc.dram_tensor("whh0_8", [512, 2 * G], F8, kind="ExternalInput")
    wih1_8d = nc.dram_tensor("wih1_8", [512, 2 * G], F8, kind="ExternalInput")
    whh1_8d = nc.dram_tensor("whh1_8", [512, 2 * G], F8, kind="ExternalInput")
    biasT0_d = nc.dram_tensor("biasT0", [128, 32], F32, kind="ExternalInput")
    biasT1_d = nc.dram_tensor("biasT1", [128, 32], F32, kind="ExternalInput")
    wencT = nc.dram_tensor("wencT", [E, J], F32R, kind="ExternalInput")
    benc = nc.dram_tensor("benc", [J], F32R, kind="ExternalInput")
    wdec8_d = nc.dram_tensor("wdec8", [D, J], F8, kind="ExternalInput")
    woutT = nc.dram_tensor("woutT", [J, O], F32R, kind="ExternalInput")
    bout_bc = nc.dram_tensor("bout_bc", [128, O], F32, kind="ExternalInput")
    ones_d = nc.dram_tensor("ones_d", [256], F32R, kind="ExternalInput")
    ident_f = nc.dram_tensor("ident_f", [128, 128], F32, kind="ExternalInput")
    sel8_d = nc.dram_tensor("sel8_d", [128, 512], F8, kind="ExternalInput")
    out = nc.dram_tensor("out", [BT * U, O], F32, kind="ExternalOutput")

    with tile.TileContext(nc) as tc, ExitStack() as es:
        cpool = es.enter_context(tc.tile_pool(name="const", bufs=1))
        ppool = es.enter_context(tc.tile_pool(name="persist", bufs=1))

        ident = cpool.tile([128, 128], F32, tag="ident", name="ident")
        nc.sync.dma_start(ident[:], ident_f[:])
        ones_r = cpool.tile([1, 256], F32R, tag="ones", name="ones")
        nc.sync.dma_start(ones_r[:1, :], ones_d[None, :])
        sel8 = cpool.tile([128, 2, 256], F8, tag="sel8", name="sel8")
        nc.sync.dma_start(sel8[:].bitcast(F32), sel8_d[:].bitcast(F32))

        # persistent state
        gxT8s = [ppool.tile([128, 64, 2, 16], F8, tag=f"gxT8s_{p}",
                            name=f"gxT8s_{p}") for p in range(16)]
        hT8s_0 = [ppool.tile([128, 64, 2, 16], F8, tag=f"hT8s0_{c}",
                             name=f"hT8s0_{c}") for c in range(KP)]
        hT8s_1 = [ppool.tile([128, 64, 2, 16], F8, tag=f"hT8s1_{c}",
                             name=f"hT8s1_{c}") for c in range(KP)]
        hT8m_0 = [ppool.tile([128, 2, 64, 4], F8, tag=f"hT8m0_{c}",
                             name=f"hT8m0_{c}") for c in range(KP)]
        zT8 = ppool.tile([128, 2, 16], F8, tag="zT8", name="zT8")
        gates = ppool.tile([4, G], F32, tag="gates", name="gates")
        c_sb = ppool.tile([4, D], F32, tag="c", name="c")
        hbuf = ppool.tile([4, D], F32, tag="h", name="h")
        nc.gpsimd.memset(zT8[:].bitcast(F32), 0.0)
        nc.gpsimd.memset(c_sb[:], 0.0)

        biasT0 = ppool.tile([128, 32], F32, tag="bT0", name="bT0")
        biasT1 = ppool.tile([128, 32], F32, tag="bT1", name="bT1")
        encp = [ppool.tile([128, BT], F32, tag=f"encp{jt}", name=f"encp{jt}")
                for jt in range(4)]

        pools = {"sel8": sel8}

        # ---- Phase 1: embedding gather -> eysT8; enc-side joint prep ----
        with tc.tile_pool(name="ph1", bufs=1) as p1, \
             tc.tile_pool(name="pT", bufs=2, space="PSUM") as pT, \
             tc.tile_pool(name="pbig", bufs=4, space="PSUM") as pbig:
            pools["pbig"] = pbig
            eysT8 = [p1.tile([128, 2, 256], F8, tag=f"ey8_{c}", name=f"ey8_{c}")
                     for c in range(2)]
            idxs = []
            for t in range(2):
                idx = p1.tile([128, 1], I32, tag=f"idx{t}", name=f"idx{t}")
                nc.sync.dma_start(idx[:, :1], ys_idx[128 * t:128 * (t + 1), None])
                idxs.append(idx)
            # weight DMAs, in rough order of first use
            wih0_8 = [p1.tile([128, 2, G], F8, tag=f"wih08_{c}", name=f"wih08_{c}")
                      for c in range(2)]
            for c in range(2):
                nc.sync.dma_start(wih0_8[c][:].bitcast(F32),
                                  wih0_8d[128 * c:128 * (c + 1), :].bitcast(F32))
            nc.sync.dma_start(biasT0[:], biasT0_d[:])
            nc.sync.dma_start(biasT1[:], biasT1_d[:])

            for t in range(2):
                ey = p1.tile([128, E], F32, tag=f"ey{t}", name=f"ey{t}")
                nc.gpsimd.indirect_dma_start(
                    out=ey[:], out_offset=None, in_=embed8[:],
                    in_offset=bass.IndirectOffsetOnAxis(ap=idxs[t][:, :1], axis=0))
                for ec in range(4):
                    tp = pT.tile([128, 128], F32, tag="pT", name="pT")
                    nc.tensor.transpose(tp[:], in_=ey[:, 128 * ec:128 * (ec + 1)],
                                        identity=ident[:])
                    nc.vector.tensor_copy(
                        eysT8[ec >> 1][:, ec & 1, 128 * t:128 * (t + 1)], tp[:])

            # enc-side: hs^T and enc_p^T (PE slack here)
            wenc = [p1.tile([128, J], F32R, tag=f"wenc{ec}", name=f"wenc{ec}")
                    for ec in range(4)]
            benc_sb = p1.tile([1, J], F32R, tag="benc", name="benc")
            hs_sb = p1.tile([BT, E], F32, tag="hs_sb", name="hs_sb")
            hsT = [p1.tile([128, BT], F32R, tag=f"hsT{ec}", name=f"hsT{ec}")
                   for ec in range(4)]
            for ec in range(4):
                nc.sync.dma_start(wenc[ec][:], wencT[128 * ec:128 * (ec + 1), :])
            nc.sync.dma_start(hs_sb[:], hs[:])
            nc.sync.dma_start(benc_sb[:1, :], benc[None, :])
            for ec in range(4):
                tp = pT.tile([128, 128], F32, tag="pT", name="pT")
                nc.tensor.transpose(tp[:, :BT], in_=hs_sb[:, 128 * ec:128 * (ec + 1)],
                                    identity=ident[:BT, :BT])
                nc.vector.tensor_copy(hsT[ec][:], tp[:, :BT])
            for jt in range(4):
                tpp = pT.tile([128, 128], F32, tag="pT", name="pT")
                pse = tpp[:, :BT]
                for ec in range(4):
                    _mm_r(nc, pse, lhsT=wenc[ec][:, 128 * jt:128 * (jt + 1)],
                          rhs=hsT[ec][:], start=(ec == 0), stop=False)
                _mm_r(nc, pse, lhsT=benc_sb[:1, 128 * jt:128 * (jt + 1)],
                      rhs=ones_r[:1, :BT], start=False, stop=True)
                nc.vector.tensor_copy(encp[jt][:], pse)

            # ---- Phase 2: layer-0 x-projection (fp8 DR) ----
            _emit_xproj8(nc, pools, wih0_8, 2, lambda c: eysT8[c][:], biasT0,
                         gxT8s)

        # ---- Phase 3: layer-0 recurrence ----
        with tc.tile_pool(name="wih1p", bufs=1) as wih1p:
            wih1_8 = [wih1p.tile([128, 2, G], F8, tag=f"wih18_{c}", name=f"wih18_{c}")
                      for c in range(KP)]
            with tc.tile_pool(name="whhA", bufs=1) as whhp, \
                 tc.tile_pool(name="pgatesA", bufs=3, space="PSUM") as pgates, \
                 tc.tile_pool(name="ptrA", bufs=2, space="PSUM") as ptr:
                whh0_8 = [whhp.tile([128, 2, G], F8, tag=f"whh08_{c}",
                                    name=f"whh08_{c}") for c in range(KP)]
                for c in range(KP):
                    nc.sync.dma_start(whh0_8[c][:].bitcast(F32),
                                      whh0_8d[128 * c:128 * (c + 1), :].bitcast(F32))
                # prefetch layer-1 weights during the recurrence
                for c in range(KP):
                    nc.sync.dma_start(wih1_8[c][:].bitcast(F32),
                                      wih1_8d[128 * c:128 * (c + 1), :].bitcast(F32))
                pools["pgates"], pools["ptr"] = pgates, ptr
                _emit_lstm_layer(nc, pools, ident[:], whh0_8, zT8, gxT8s,
                                 hT8s_0, c_sb, gates, hbuf, steps)

            # ---- Phase 4: layer-1 x-projection (fp8 DR) ----
            # bulk-derive the moving layout hT8m_0 from hT8s_0 (strided copies)
            for c in range(KP):
                for i in range(2):
                    for m in range(4):
                        nc.vector.tensor_copy(hT8m_0[c][:, i, :, m],
                                              hT8s_0[c][:, :, i, m])
            with tc.tile_pool(name="pbig2", bufs=4, space="PSUM") as pbig2:
                pools["pbig"] = pbig2
                _emit_xproj8(nc, pools, wih1_8, KP,
                             lambda c: hT8m_0[c][:, :, :, :], biasT1, gxT8s)

        # ---- Phase 5: layer-1 recurrence ----
        nc.gpsimd.memset(c_sb[:], 0.0)
        with tc.tile_pool(name="jointw", bufs=1) as jwp:
            wdec8 = [jwp.tile([128, J], F8, tag=f"wdec8_{k}", name=f"wdec8_{k}")
                     for k in range(8)]
            wout = [jwp.tile([128, O], F32R, tag=f"wout{jt}", name=f"wout{jt}")
                    for jt in range(4)]
            bout_sb = jwp.tile([128, O], F32, tag="bout", name="bout")
            with tc.tile_pool(name="whhB", bufs=1) as whhp2, \
                 tc.tile_pool(name="pgatesB", bufs=3, space="PSUM") as pgates2, \
                 tc.tile_pool(name="ptrB", bufs=2, space="PSUM") as ptr2:
                whh1_8 = [whhp2.tile([128, 2, G], F8, tag=f"whh18_{c}",
                                     name=f"whh18_{c}") for c in range(KP)]
                for c in range(KP):
                    nc.sync.dma_start(whh1_8[c][:].bitcast(F32),
                                      whh1_8d[128 * c:128 * (c + 1), :].bitcast(F32))
                # prefetch joint weights during the recurrence
                for k in range(8):
                    nc.sync.dma_start(wdec8[k][:].bitcast(F32),
                                      wdec8_d[128 * k:128 * (k + 1), :].bitcast(F32))
                for jt in range(4):
                    nc.sync.dma_start(wout[jt][:],
                                      woutT[128 * jt:128 * (jt + 1), :])
                nc.sync.dma_start(bout_sb[:], bout_bc[:])

                pools["pgates"], pools["ptr"] = pgates2, ptr2
                _emit_lstm_layer(nc, pools, ident[:], whh1_8, zT8, gxT8s,
                                 hT8s_1, c_sb, gates, hbuf, steps)

            # ---- Phase 6: joint network on this core's T-slice ----
            with tc.tile_pool(name="decp", bufs=1) as dpp, \
                 tc.tile_pool(name="zt", bufs=4) as ztp, \
                 tc.tile_pool(name="osb", bufs=3) as osbp, \
                 tc.tile_pool(name="pj", bufs=2, space="PSUM") as pj:
                decp = [dpp.tile([128, 256], F32, tag=f"decp{jt}", name=f"decp{jt}")
                        for jt in range(4)]
                # dec_p^T (unscaled), columns (b, u), read from hT8s_1 directly
                for jt in range(4):
                    ps = pj.tile([128, 2048], F32, tag="pj", name="pj")
                    for k in range(8):
                        rhs = hT8s_1[k >> 1][:, :, k & 1, 0:4].rearrange(
                            "p u b -> p b u", u=U, b=B)
                        nc.tensor.matmul(
                            ps[:, 0:256], lhsT=wdec8[k][:, 128 * jt:128 * (jt + 1)],
                            rhs=rhs, start=(k == 0), stop=(k == 7))
                    nc.vector.tensor_scalar(decp[jt][:], ps[:, 0:256], 1.0 / SDEC,
                                            None, OP.mult)
                # z^T tiles + output matmul, one M-tile (=2 encoder rows) at a time
                for m in range(BT * U // 128):
                    zt = [ztp.tile([128, 128], F32R, tag=f"zt{jt}", name=f"zt{jt}")
                          for jt in range(4)]
                    for jt in range(4):
                        for half in range(2):
                            bt = 2 * m + half
                            b = bt // TC
                            nc.scalar.activation(
                                zt[jt][:, half * 64:(half + 1) * 64],
                                decp[jt][:, b * 64:(b + 1) * 64],
                                AF.Tanh, bias=encp[jt][:, bt:bt + 1])
                    ps = pj.tile([128, 2048], F32, tag="pj", name="pj")
                    for ob in range(4):
                        obs = slice(ob * 512, (ob + 1) * 512)
                        for jt in range(4):
                            _mm_r(nc, ps[:, obs], lhsT=zt[jt][:],
                                  rhs=wout[jt][:, obs],
                                  start=(jt == 0), stop=(jt == 3))
                    o_sb = osbp.tile([128, O], F32, tag="osb", name="osb")
                    nc.vector.tensor_tensor(o_sb[:], in0=ps[:], in1=bout_sb[:],
                                            op=OP.add)
                    nc.sync.dma_start(out[128 * m:128 * (m + 1), :], o_sb[:])

    nc.compile()
    return nc


def _dr_pack(Wt, scale):
    """Wt [K, N] f32 -> DoubleRow-interleaved fp8 [K//2, 2N].

    Row (c, p), cols (i, n): value = scale * Wt[256c + 128i + p, n].
    """
    K, N = Wt.shape
    w = np.asarray(Wt * scale, np.float32).astype(E4NP)
    return np.ascontiguousarray(
        w.reshape(K // 256, 2, 128, N).transpose(0, 2, 1, 3).reshape(K // 2, 2 * N))


def _prep_inputs(hs_pad, ys_in_pad, embed, W_ih0, W_hh0, b_ih0, b_hh0,
                 W_ih1, W_hh1, b_ih1, b_hh1, W_enc, b_enc, W_dec, W_out, b_out):
    f = np.float32
    tr = lambda a: np.ascontiguousarray(np.asarray(a).T, dtype=f)
    sel = np.zeros((128, 512), E4NP)
    for p in range(128):
        sel[p, p] = 1
        sel[p, 256 + 128 + p] = 1
    b0 = (np.asarray(b_ih0, f) + np.asarray(b_hh0, f)) * SW
    b1 = (np.asarray(b_ih1, f) + np.asarray(b_hh1, f)) * SW
    common = {
        "ys_idx": np.ascontiguousarray(np.asarray(ys_in_pad).T.reshape(-1),
                                       dtype=np.int32),
        "embed8": np.ascontiguousarray(np.asarray(embed, f) * 8.0),
        "wih0_8": _dr_pack(tr(W_ih0), 4.0),
        "whh0_8": _dr_pack(tr(W_hh0), SW),
        "wih1_8": _dr_pack(tr(W_ih1), SW),
        "whh1_8": _dr_pack(tr(W_hh1), SW),
        "biasT0": np.ascontiguousarray(b0.reshape(32, 128).T),
        "biasT1": np.ascontiguousarray(b1.reshape(32, 128).T),
        "wencT": tr(W_enc),
        "benc": np.asarray(b_enc, f),
        "wdec8": np.ascontiguousarray((tr(W_dec) * SDEC).astype(E4NP)),
        "woutT": tr(W_out),
        "bout_bc": np.ascontiguousarray(
            np.broadcast_to(np.asarray(b_out, f)[None, :], (128, O))),
        "ones_d": np.ones(256, f),
        "ident_f": np.eye(128, dtype=f),
        "sel8_d": sel,
    }
    hs_np = np.asarray(hs_pad, f)
    in_maps = []
    for c in range(NCORES):
        m = dict(common)
        m["hs"] = np.ascontiguousarray(
            hs_np[:, c * TC:(c + 1) * TC, :].reshape(BT, E))
        in_maps.append(m)
    return in_maps


def _get_runner():
    """Build (once) a reusable jitted SPMD callable (weights replicated)."""
    if "runner" in _CACHE:
        return _CACHE["runner"]
    import jax
    from jax.sharding import Mesh, PartitionSpec as P
    from jax.experimental.shard_map import shard_map
    from concourse import bass2jax
    import concourse.mybir as mybir_

    nc = _CACHE.get("nc")
    if nc is None:
        nc = _CACHE["nc"] = build_nc()
    bass2jax.install_neuronx_cc_hook()

    pname = nc.partition_id_tensor.name if nc.partition_id_tensor else None
    in_names, out_names, out_avals = [], [], []
    for alloc in nc.m.functions[0].allocations:
        if not isinstance(alloc, mybir_.MemoryLocationSet):
            continue
        name = alloc.memorylocations[0].name
        if alloc.kind == "ExternalInput":
            if name != pname:
                in_names.append(name)
        elif alloc.kind == "ExternalOutput":
            out_names.append(name)
            shape = tuple(alloc.tensor_shape)
            out_avals.append(jax.core.ShapedArray(shape, mybir_.dt.np(alloc.dtype)))
    n_params = len(in_names)
    all_names = in_names + out_names
    if pname is not None:
        all_names = all_names + [pname]

    def _body(*args):
        operands = list(args)
        if pname is not None:
            operands.append(bass2jax.partition_id_tensor())
        outs = bass2jax._bass_exec_p.bind(
            *operands,
            out_avals=tuple(out_avals),
            in_names=tuple(all_names),
            out_names=tuple(out_names),
            lowering_input_output_aliases=(),
            sim_require_finite=True,
            sim_require_nnan=True,
            nc=nc,
        )
        return tuple(outs)

    devices = jax.devices()[:NCORES]
    mesh = Mesh(np.asarray(devices), ("core",))
    in_specs = tuple(P("core") if n == "hs" else P() for n in in_names)
    in_specs = in_specs + (P("core"),) * len(out_names)
    out_specs = (P("core"),) * len(out_names)
    fn = jax.jit(shard_map(_body, mesh=mesh, in_specs=in_specs,
                           out_specs=out_specs, check_rep=False))

    def _chain(n):
        def body_n(*args):
            ins, outbuf = args[:n_params], args[n_params]
            for _ in range(n):
                (outbuf,) = _body(*ins, outbuf)
            return (outbuf,)
        return jax.jit(shard_map(body_n, mesh=mesh, in_specs=in_specs,
                                 out_specs=out_specs, check_rep=False))

    runner = (fn, in_names, out_names, out_avals, mesh, _chain)
    _CACHE["runner"] = runner
    return runner


def _device_args(in_maps):
    fn, in_names, out_names, out_avals, mesh, _chain = _get_runner()
    args = []
    for n in in_names:
        if n == "hs":
            args.append(np.concatenate([m["hs"] for m in in_maps], axis=0))
        else:
            args.append(in_maps[0][n])
    for av in out_avals:
        args.append(np.zeros((NCORES * av.shape[0],) + av.shape[1:], av.dtype))
    return args


def kernel(**inputs) -> np.ndarray:
    fn, in_names, out_names, out_avals, mesh, _chain = _get_runner()
    in_maps = _prep_inputs(**inputs)
    args = _device_args(in_maps)
    outs = fn(*args)
    out = np.asarray(outs[0])  # (8*4096, 2048)
    return out.reshape(NCORES, B, TC, U, O).transpose(1, 0, 2, 3, 4).reshape(B, T, U, O)


if __name__ == "__main__":
    import time
    t0 = time.time()
    nc = build_nc(steps=int(sys.argv[1]) if len(sys.argv) > 1 else U)
    print(f"built ok in {time.time()-t0:.1f}s", flush=True)


# revision 39
# speedup vs baseline: 1.1289x; 1.1289x over previous
"""RNN-T decoder (embedding + 2-layer LSTM + joint network) on 8 Trainium2 cores.

Strategy (v2, fp8 DoubleRow):
  - LSTM (B=4, U=64, D=1024) runs redundantly on all 8 cores. The recurrent
    matmul streams W_hh (pre-scaled x32, fp8e4m3, DoubleRow-interleaved) through
    the PE at 0.5 cyc/row with the tiny h^T chunk (fp8) stationary. The batched
    x-projections are fp8 DoubleRow too, injected into the gate PSUM via a
    constant selector matrix. Activation scale 1/32 undoes the weight scaling.
  - Tokens are b-major (tok = 64*b + u) so every hT/gxT consumer is a clean
    contiguous or 2-level-strided AP.
  - Joint network (dominant FLOPs) is sharded over T: core c computes
    out[:, c*16:(c+1)*16, :, :]. Stays f32r for precision; dec_p uses fp8
    wdec (x16) with the scale folded into the z-tanh activation.

kernel(**inputs) takes the full unsharded inputs (as in reference.setup_inputs)
and returns the full (B, T, U, ODIM) float32 output.
"""
import sys
import numpy as np
import ml_dtypes

sys.path.insert(0, "/opt/trn_rl_repo")

import concourse.bass as bass
import concourse.bacc as bacc
import concourse.mybir as mybir
import concourse.tile as tile
from contextlib import ExitStack

F32 = mybir.dt.float32
F32R = mybir.dt.float32r
BF16 = mybir.dt.bfloat16
F8 = mybir.dt.float8e4
I32 = mybir.dt.int32
AF = mybir.ActivationFunctionType
OP = mybir.AluOpType
DRM = mybir.MatmulPerfMode.DoubleRow
E4NP = ml_dtypes.float8_e4m3

B, T, U = 4, 128, 64
NCORES = 8
TC = T // NCORES           # 16 T-columns per core
E, D, G = 512, 1024, 4096  # embed, dunits, 4*dunits
J, O = 512, 2048           # joint dim, odim
UB = U * B                 # 256 tokens, b-major: tok = 64*b + u
BT = B * TC                # 64 encoder rows per core
KP = D // 256              # 4 contraction pairs of the hidden dim
SW = 32.0                  # gate pre-activation scale (weights x32)
SDEC = 16.0                # wdec scale

_CACHE = {}


def _mm_r(nc, out, lhsT, rhs, **kw):
    nc.tensor.matmul(out, lhsT=lhsT.bitcast(F32R), rhs=rhs.bitcast(F32R), **kw)


def _mm8(nc, out, lhsT, rhs, start, stop):
    nc.tensor.matmul(out, lhsT=lhsT, rhs=rhs, start=start, stop=stop,
                     perf_mode=DRM)


def _emit_xproj8(nc, pools, w8, nchunks, rhs_of_c, biasT, gxT8s):
    """gxT8s[pc][:, :, gm&1, 0:4] = fp8( (W x)^T + bias ), 32 gate-chunks.

    w8: list of [128, 2, G] fp8 tiles (DR-interleaved W.T, pre-scaled x32)
    rhs_of_c: c -> [128, 2, 256] fp8 moving operand (x^T pair chunk, u-major)
    biasT: [128, 32] f32 tile; bias for gate-chunk gm is biasT[:, gm:gm+1]
    gxT8s: 16 tiles [128, 64, 2, 16] fp8 (aligned stationary layout)
    """
    pbig = pools["pbig"]
    for gm in range(32):
        ps = pbig.tile([128, 256], F32, tag="pbig", name="pbig")
        for c in range(nchunks):
            _mm8(nc, ps[:], w8[c][:, :, 128 * gm:128 * (gm + 1)], rhs_of_c(c),
                 start=(c == 0), stop=(c == nchunks - 1))
        # PSUM -> fp8 gxT8s with per-partition bias; DVE (ACT-table free)
        nc.vector.tensor_scalar(gxT8s[gm >> 1][:, :, gm & 1, 0:4], ps[:],
                                biasT[:, gm:gm + 1], None, OP.add)


def _emit_lstm_layer(nc, pools, ident, whh8, zT8, gxT8s, hT8s, c_sb, gates,
                     hbuf, steps):
    """One LSTM layer, `steps` sequential steps, fp8 DoubleRow h-matmuls.

    All activations use the Tanh table only (sigmoid via 0.5*tanh(x/2)+0.5,
    fixup on Pool/DVE) to avoid ACT table switching.

    whh8:  4 tiles [128, 2, G] fp8 = W_hh.T DR pairs (moving operand, x32)
    gxT8s: 16 tiles [128, 64, 2, 16] fp8 = (W_ih x + b)^T x32, aligned
           stationary layout for DR injects via the sel8 selector
    hT8s:  4 tiles [128, 64, 2, 16] fp8; [c][:, u, i, m] = h_u[m, 256c+128i+p]
           written at slot u after step u; step u reads slot u-1 (zT8 u=0).
    """
    pgates, ptr = pools["pgates"], pools["ptr"]
    sel8 = pools["sel8"]
    ident4 = ident[:4, :4]
    for u in range(steps):
        for gi in (2, 1, 0, 3):  # g, f, i, o
            gs = slice(1024 * gi, 1024 * (gi + 1))
            ps = pgates.tile([4, 1024], F32, tag="pg", name="pg")
            for pc4 in range(4):
                # start on the first write of each 2KB PSUM bank (zero region)
                nc.tensor.matmul(
                    ps[:, 256 * pc4:256 * (pc4 + 1)],
                    lhsT=gxT8s[4 * gi + pc4][:, u, :, 0:4], rhs=sel8[:],
                    start=(pc4 in (0, 2)), stop=False, perf_mode=DRM)
            for c in range(4):
                lt = zT8[:, :, 0:4] if u == 0 else hT8s[c][:, u - 1, :, 0:4]
                for hf in range(2):
                    nc.tensor.matmul(
                        ps[:, 512 * hf:512 * (hf + 1)], lhsT=lt,
                        rhs=whh8[c][:, :, 1024 * gi + 512 * hf:
                                    1024 * gi + 512 * (hf + 1)],
                        start=False, stop=(c == 3), perf_mode=DRM)
            fn = AF.Tanh if gi == 2 else AF.Sigmoid
            nc.scalar.activation(gates[:, gs], ps[:], fn, scale=1.0 / SW)
        # c' = sig(f)*c + sig(i)*tanh(g);  h = sig(o)*tanh(c')
        for hh in range(2):
            s = slice(512 * hh, 512 * (hh + 1))
            gi_ = gates[:, 0 * D:][:, s]
            gf = gates[:, 1 * D:][:, s]
            gg = gates[:, 2 * D:][:, s]
            go = gates[:, 3 * D:][:, s]
            ch = c_sb[:, s]
            nc.vector.tensor_tensor(gf, in0=gf, in1=ch, op=OP.mult)    # f*c
            nc.gpsimd.tensor_tensor(gi_, in0=gi_, in1=gg, op=OP.mult)  # i*g~
            nc.gpsimd.tensor_tensor(ch, in0=gf, in1=gi_, op=OP.add)    # c'
            nc.scalar.activation(gg, ch, AF.Tanh)                      # tanh(c')
            nc.vector.tensor_tensor(hbuf[:, s], in0=go, in1=gg, op=OP.mult)
            for k in range(4 * hh, 4 * hh + 4):
                tp = ptr.tile([128, 4], F32, tag="tr", name="tr")
                nc.tensor.transpose(tp[:], in_=hbuf[:, 128 * k:128 * (k + 1)],
                                    identity=ident4)
                # Pool/GPSIMD cannot read PSUM -> copy on DVE
                nc.vector.tensor_copy(hT8s[k >> 1][:, u, k & 1, 0:4], tp[:])


def build_nc(steps=U):
    nc = bacc.Bacc("TRN2", target_bir_lowering=False, debug=False)

    hs = nc.dram_tensor("hs", [BT, E], F32, kind="ExternalInput")
    ys_idx = nc.dram_tensor("ys_idx", [UB], I32, kind="ExternalInput")
    embed8 = nc.dram_tensor("embed8", [O, E], F32, kind="ExternalInput")
    wih0_8d = nc.dram_tensor("wih0_8", [256, 2 * G], F8, kind="ExternalInput")
    whh0_8d = nc.dram_tensor("whh0_8", [512, 2 * G], F8, kind="ExternalInput")
    wih1_8d = nc.dram_tensor("wih1_8", [512, 2 * G], F8, kind="ExternalInput")
    whh1_8d = nc.dram_tensor("whh1_8", [512, 2 * G], F8, kind="ExternalInput")
    biasT0_d = nc.dram_tensor("biasT0", [128, 32], F32, kind="ExternalInput")
    biasT1_d = nc.dram_tensor("biasT1", [128, 32], F32, kind="ExternalInput")
    wencT = nc.dram_tensor("wencT", [E, J], F32R, kind="ExternalInput")
    benc = nc.dram_tensor("benc", [J], F32R, kind="ExternalInput")
    wdec8_d = nc.dram_tensor("wdec8", [D, J], F8, kind="ExternalInput")
    woutT = nc.dram_tensor("woutT", [J, O], F32R, kind="ExternalInput")
    bout_bc = nc.dram_tensor("bout_bc", [128, O], F32, kind="ExternalInput")
    ones_d = nc.dram_tensor("ones_d", [256], F32R, kind="ExternalInput")
    ident_f = nc.dram_tensor("ident_f", [128, 128], F32, kind="ExternalInput")
    sel8_d = nc.dram_tensor("sel8_d", [128, 512], F8, kind="ExternalInput")
    out = nc.dram_tensor("out", [BT * U, O], F32, kind="ExternalOutput")

    with tile.TileContext(nc) as tc, ExitStack() as es:
        cpool = es.enter_context(tc.tile_pool(name="const", bufs=1))
        ppool = es.enter_context(tc.tile_pool(name="persist", bufs=1))

        ident = cpool.tile([128, 128], F32, tag="ident", name="ident")
        nc.sync.dma_start(ident[:], ident_f[:])
        ones_r = cpool.tile([1, 256], F32R, tag="ones", name="ones")
        nc.sync.dma_start(ones_r[:1, :], ones_d[None, :])
        sel8 = cpool.tile([128, 2, 256], F8, tag="sel8", name="sel8")
        nc.sync.dma_start(sel8[:].bitcast(F32), sel8_d[:].bitcast(F32))

        # persistent state
        gxT8s = [ppool.tile([128, 64, 2, 16], F8, tag=f"gxT8s_{p}",
                            name=f"gxT8s_{p}") for p in range(16)]
        hT8s_0 = [ppool.tile([128, 64, 2, 16], F8, tag=f"hT8s0_{c}",
                             name=f"hT8s0_{c}") for c in range(KP)]
        hT8s_1 = [ppool.tile([128, 64, 2, 16], F8, tag=f"hT8s1_{c}",
                             name=f"hT8s1_{c}") for c in range(KP)]
        hT8m_0 = [ppool.tile([128, 2, 64, 4], F8, tag=f"hT8m0_{c}",
                             name=f"hT8m0_{c}") for c in range(KP)]
        zT8 = ppool.tile([128, 2, 16], F8, tag="zT8", name="zT8")
        gates = ppool.tile([4, G], F32, tag="gates", name="gates")
        c_sb = ppool.tile([4, D], F32, tag="c", name="c")
        hbuf = ppool.tile([4, D], F32, tag="h", name="h")
        nc.gpsimd.memset(zT8[:].bitcast(F32), 0.0)
        nc.gpsimd.memset(c_sb[:], 0.0)

        biasT0 = ppool.tile([128, 32], F32, tag="bT0", name="bT0")
        biasT1 = ppool.tile([128, 32], F32, tag="bT1", name="bT1")
        encp = [ppool.tile([128, BT], F32, tag=f"encp{jt}", name=f"encp{jt}")
                for jt in range(4)]

        pools = {"sel8": sel8}

        # ---- Phase 1: embedding gather -> eysT8; enc-side joint prep ----
        with tc.tile_pool(name="ph1", bufs=1) as p1, \
             tc.tile_pool(name="pT", bufs=2, space="PSUM") as pT, \
             tc.tile_pool(name="pbig", bufs=4, space="PSUM") as pbig:
            pools["pbig"] = pbig
            eysT8 = [p1.tile([128, 2, 256], F8, tag=f"ey8_{c}", name=f"ey8_{c}")
                     for c in range(2)]
            idxs = []
            for t in range(2):
                idx = p1.tile([128, 1], I32, tag=f"idx{t}", name=f"idx{t}")
                nc.sync.dma_start(idx[:, :1], ys_idx[128 * t:128 * (t + 1), None])
                idxs.append(idx)
            # weight DMAs, in rough order of first use
            wih0_8 = [p1.tile([128, 2, G], F8, tag=f"wih08_{c}", name=f"wih08_{c}")
                      for c in range(2)]
            for c in range(2):
                nc.sync.dma_start(wih0_8[c][:].bitcast(F32),
                                  wih0_8d[128 * c:128 * (c + 1), :].bitcast(F32))
            nc.sync.dma_start(biasT0[:], biasT0_d[:])
            nc.sync.dma_start(biasT1[:], biasT1_d[:])

            for t in range(2):
                ey = p1.tile([128, E], F32, tag=f"ey{t}", name=f"ey{t}")
                nc.gpsimd.indirect_dma_start(
                    out=ey[:], out_offset=None, in_=embed8[:],
                    in_offset=bass.IndirectOffsetOnAxis(ap=idxs[t][:, :1], axis=0))
                for ec in range(4):
                    tp = pT.tile([128, 128], F32, tag="pT", name="pT")
                    nc.tensor.transpose(tp[:], in_=ey[:, 128 * ec:128 * (ec + 1)],
                                        identity=ident[:])
                    nc.vector.tensor_copy(
                        eysT8[ec >> 1][:, ec & 1, 128 * t:128 * (t + 1)], tp[:])

            # enc-side: hs^T and enc_p^T (PE slack here)
            wenc = [p1.tile([128, J], F32R, tag=f"wenc{ec}", name=f"wenc{ec}")
                    for ec in range(4)]
            benc_sb = p1.tile([1, J], F32R, tag="benc", name="benc")
            hs_sb = p1.tile([BT, E], F32, tag="hs_sb", name="hs_sb")
            hsT = [p1.tile([128, BT], F32R, tag=f"hsT{ec}", name=f"hsT{ec}")
                   for ec in range(4)]
            for ec in range(4):
                nc.sync.dma_start(wenc[ec][:], wencT[128 * ec:128 * (ec + 1), :])
            nc.sync.dma_start(hs_sb[:], hs[:])
            nc.sync.dma_start(benc_sb[:1, :], benc[None, :])
            for ec in range(4):
                tp = pT.tile([128, 128], F32, tag="pT", name="pT")
                nc.tensor.transpose(tp[:, :BT], in_=hs_sb[:, 128 * ec:128 * (ec + 1)],
                                    identity=ident[:BT, :BT])
                nc.vector.tensor_copy(hsT[ec][:], tp[:, :BT])
            for jt in range(4):
                tpp = pT.tile([128, 128], F32, tag="pT", name="pT")
                pse = tpp[:, :BT]
                for ec in range(4):
                    _mm_r(nc, pse, lhsT=wenc[ec][:, 128 * jt:128 * (jt + 1)],
                          rhs=hsT[ec][:], start=(ec == 0), stop=False)
                _mm_r(nc, pse, lhsT=benc_sb[:1, 128 * jt:128 * (jt + 1)],
                      rhs=ones_r[:1, :BT], start=False, stop=True)
                nc.vector.tensor_copy(encp[jt][:], pse)

            # ---- Phase 2: layer-0 x-projection (fp8 DR) ----
            _emit_xproj8(nc, pools, wih0_8, 2, lambda c: eysT8[c][:], biasT0,
                         gxT8s)

        # ---- Phase 3: layer-0 recurrence ----
        with tc.tile_pool(name="wih1p", bufs=1) as wih1p:
            wih1_8 = [wih1p.tile([128, 2, G], F8, tag=f"wih18_{c}", name=f"wih18_{c}")
                      for c in range(KP)]
            with tc.tile_pool(name="whhA", bufs=1) as whhp, \
                 tc.tile_pool(name="pgatesA", bufs=3, space="PSUM") as pgates, \
                 tc.tile_pool(name="ptrA", bufs=2, space="PSUM") as ptr:
                whh0_8 = [whhp.tile([128, 2, G], F8, tag=f"whh08_{c}",
                                    name=f"whh08_{c}") for c in range(KP)]
                for c in range(KP):
                    nc.sync.dma_start(whh0_8[c][:].bitcast(F32),
                                      whh0_8d[128 * c:128 * (c + 1), :].bitcast(F32))
                # prefetch layer-1 weights during the recurrence
                for c in range(KP):
                    nc.sync.dma_start(wih1_8[c][:].bitcast(F32),
                                      wih1_8d[128 * c:128 * (c + 1), :].bitcast(F32))
                pools["pgates"], pools["ptr"] = pgates, ptr
                _emit_lstm_layer(nc, pools, ident[:], whh0_8, zT8, gxT8s,
                                 hT8s_0, c_sb, gates, hbuf, steps)

            # ---- Phase 4: layer-1 x-projection (fp8 DR) ----
            # bulk-derive the moving layout hT8m_0 from hT8s_0 (strided copies)
            for c in range(KP):
                for i in range(2):
                    for m in range(4):
                        nc.vector.tensor_copy(hT8m_0[c][:, i, :, m],
                                              hT8s_0[c][:, :, i, m])
            with tc.tile_pool(name="pbig2", bufs=4, space="PSUM") as pbig2:
                pools["pbig"] = pbig2
                _emit_xproj8(nc, pools, wih1_8, KP,
                             lambda c: hT8m_0[c][:, :, :, :], biasT1, gxT8s)

        # ---- Phase 5: layer-1 recurrence ----
        nc.gpsimd.memset(c_sb[:], 0.0)
        with tc.tile_pool(name="jointw", bufs=1) as jwp:
            wdec8 = [jwp.tile([128, J], F8, tag=f"wdec8_{k}", name=f"wdec8_{k}")
                     for k in range(8)]
            wout = [jwp.tile([128, O], F32R, tag=f"wout{jt}", name=f"wout{jt}")
                    for jt in range(4)]
            bout_sb = jwp.tile([128, O], F32, tag="bout", name="bout")
            with tc.tile_pool(name="whhB", bufs=1) as whhp2, \
                 tc.tile_pool(name="pgatesB", bufs=3, space="PSUM") as pgates2, \
                 tc.tile_pool(name="ptrB", bufs=2, space="PSUM") as ptr2:
                whh1_8 = [whhp2.tile([128, 2, G], F8, tag=f"whh18_{c}",
                                     name=f"whh18_{c}") for c in range(KP)]
                for c in range(KP):
                    nc.sync.dma_start(whh1_8[c][:].bitcast(F32),
                                      whh1_8d[128 * c:128 * (c + 1), :].bitcast(F32))
                # prefetch joint weights during the recurrence
                for k in range(8):
                    nc.sync.dma_start(wdec8[k][:].bitcast(F32),
                                      wdec8_d[128 * k:128 * (k + 1), :].bitcast(F32))
                for jt in range(4):
                    nc.sync.dma_start(wout[jt][:],
                                      woutT[128 * jt:128 * (jt + 1), :])
                nc.sync.dma_start(bout_sb[:], bout_bc[:])

                pools["pgates"], pools["ptr"] = pgates2, ptr2
                _emit_lstm_layer(nc, pools, ident[:], whh1_8, zT8, gxT8s,
                                 hT8s_1, c_sb, gates, hbuf, steps)

            # ---- Phase 6: joint network on this core's T-slice ----
            with tc.tile_pool(name="decp", bufs=1) as dpp, \
                 tc.tile_pool(name="zt", bufs=4) as ztp, \
                 tc.tile_pool(name="zs", bufs=8) as zsp, \
                 tc.tile_pool(name="osb", bufs=4) as osbp, \
                 tc.tile_pool(name="pj", bufs=4, space="PSUM") as pj:
                decp = [dpp.tile([128, 256], F32, tag=f"decp{jt}", name=f"decp{jt}")
                        for jt in range(4)]
                # dec_p^T (unscaled), columns (b, u), read from hT8s_1 directly
                for jt in range(4):
                    ps = pj.tile([128, 256], F32, tag="pj", name="pj")
                    for k in range(8):
                        rhs = hT8s_1[k >> 1][:, :, k & 1, 0:4].rearrange(
                            "p u b -> p b u", u=U, b=B)
                        nc.tensor.matmul(
                            ps[:], lhsT=wdec8[k][:, 128 * jt:128 * (jt + 1)],
                            rhs=rhs, start=(k == 0), stop=(k == 7))
                    nc.vector.tensor_scalar(decp[jt][:], ps[:], 1.0 / SDEC,
                                            None, OP.mult)
                # z^T tiles + output matmul, one M-tile (=2 encoder rows) at a time
                for m in range(BT * U // 128):
                    zt = [ztp.tile([128, 128], F32R, tag=f"zt{jt}", name=f"zt{jt}")
                          for jt in range(4)]
                    for jt in range(4):
                        for half in range(2):
                            bt = 2 * m + half
                            b = bt // TC
                            nc.scalar.activation(
                                zt[jt][:, half * 64:(half + 1) * 64],
                                decp[jt][:, b * 64:(b + 1) * 64],
                                AF.Tanh, bias=encp[jt][:, bt:bt + 1])
                    for ob in range(4):
                        obs = slice(ob * 512, (ob + 1) * 512)
                        ps = pj.tile([128, 512], F32, tag="pj", name="pj")
                        for jt in range(4):
                            _mm_r(nc, ps[:], lhsT=zt[jt][:], rhs=wout[jt][:, obs],
                                  start=(jt == 0), stop=(jt == 3))
                        o_sb = osbp.tile([128, 512], F32, tag="osb", name="osb")
                        nc.vector.tensor_tensor(o_sb[:], in0=ps[:],
                                                in1=bout_sb[:, obs], op=OP.add)
                        nc.sync.dma_start(out[128 * m:128 * (m + 1), obs], o_sb[:])

    nc.compile()
    return nc


def _dr_pack(Wt, scale):
    """Wt [K, N] f32 -> DoubleRow-interleaved fp8 [K//2, 2N].

    Row (c, p), cols (i, n): value = scale * Wt[256c + 128i + p, n].
    """
    K, N = Wt.shape
    w = np.asarray(Wt * scale, np.float32).astype(E4NP)
    return np.ascontiguousarray(
        w.reshape(K // 256, 2, 128, N).transpose(0, 2, 1, 3).reshape(K // 2, 2 * N))


def _prep_inputs(hs_pad, ys_in_pad, embed, W_ih0, W_hh0, b_ih0, b_hh0,
                 W_ih1, W_hh1, b_ih1, b_hh1, W_enc, b_enc, W_dec, W_out, b_out):
    f = np.float32
    tr = lambda a: np.ascontiguousarray(np.asarray(a).T, dtype=f)
    sel = np.zeros((128, 512), E4NP)
    for p in range(128):
        sel[p, p] = 1
        sel[p, 256 + 128 + p] = 1
    b0 = (np.asarray(b_ih0, f) + np.asarray(b_hh0, f)) * SW
    b1 = (np.asarray(b_ih1, f) + np.asarray(b_hh1, f)) * SW
    common = {
        "ys_idx": np.ascontiguousarray(np.asarray(ys_in_pad).T.reshape(-1),
                                       dtype=np.int32),
        "embed8": np.ascontiguousarray(np.asarray(embed, f) * 8.0),
        "wih0_8": _dr_pack(tr(W_ih0), 4.0),
        "whh0_8": _dr_pack(tr(W_hh0), SW),
        "wih1_8": _dr_pack(tr(W_ih1), SW),
        "whh1_8": _dr_pack(tr(W_hh1), SW),
        "biasT0": np.ascontiguousarray(b0.reshape(32, 128).T),
        "biasT1": np.ascontiguousarray(b1.reshape(32, 128).T),
        "wencT": tr(W_enc),
        "benc": np.asarray(b_enc, f),
        "wdec8": np.ascontiguousarray((tr(W_dec) * SDEC).astype(E4NP)),
        "woutT": tr(W_out),
        "bout_bc": np.ascontiguousarray(
            np.broadcast_to(np.asarray(b_out, f)[None, :], (128, O))),
        "ones_d": np.ones(256, f),
        "ident_f": np.eye(128, dtype=f),
        "sel8_d": sel,
    }
    hs_np = np.asarray(hs_pad, f)
    in_maps = []
    for c in range(NCORES):
        m = dict(common)
        m["hs"] = np.ascontiguousarray(
            hs_np[:, c * TC:(c + 1) * TC, :].reshape(BT, E))
        in_maps.append(m)
    return in_maps


def _get_runner():
    """Build (once) a reusable jitted SPMD callable (weights replicated)."""
    if "runner" in _CACHE:
        return _CACHE["runner"]
    import jax
    from jax.sharding import Mesh, PartitionSpec as P
    from jax.experimental.shard_map import shard_map
    from concourse import bass2jax
    import concourse.mybir as mybir_

    nc = _CACHE.get("nc")
    if nc is None:
        nc = _CACHE["nc"] = build_nc()
    bass2jax.install_neuronx_cc_hook()

    pname = nc.partition_id_tensor.name if nc.partition_id_tensor else None
    in_names, out_names, out_avals = [], [], []
    for alloc in nc.m.functions[0].allocations:
        if not isinstance(alloc, mybir_.MemoryLocationSet):
            continue
        name = alloc.memorylocations[0].name
        if alloc.kind == "ExternalInput":
            if name != pname:
                in_names.append(name)
        elif alloc.kind == "ExternalOutput":
            out_names.append(name)
            shape = tuple(alloc.tensor_shape)
            out_avals.append(jax.core.ShapedArray(shape, mybir_.dt.np(alloc.dtype)))
    n_params = len(in_names)
    all_names = in_names + out_names
    if pname is not None:
        all_names = all_names + [pname]

    def _body(*args):
        operands = list(args)
        if pname is not None:
            operands.append(bass2jax.partition_id_tensor())
        outs = bass2jax._bass_exec_p.bind(
            *operands,
            out_avals=tuple(out_avals),
            in_names=tuple(all_names),
            out_names=tuple(out_names),
            lowering_input_output_aliases=(),
            sim_require_finite=True,
            sim_require_nnan=True,
            nc=nc,
        )
        return tuple(outs)

    devices = jax.devices()[:NCORES]
    mesh = Mesh(np.asarray(devices), ("core",))
    in_specs = tuple(P("core") if n == "hs" else P() for n in in_names)
    in_specs = in_specs + (P("core"),) * len(out_names)
    out_specs = (P("core"),) * len(out_names)
    fn = jax.jit(shard_map(_body, mesh=mesh, in_specs=in_specs,
                           out_specs=out_specs, check_rep=False))

    def _chain(n):
        def body_n(*args):
            ins, outbuf = args[:n_params], args[n_params]
            for _ in range(n):
                (outbuf,) = _body(*ins, outbuf)
            return (outbuf,)
        return jax.jit(shard_map(body_n, mesh=mesh, in_specs=in_specs,
                                 out_specs=out_specs, check_rep=False))

    runner = (fn, in_names, out_names, out_avals, mesh, _chain)
    _CACHE["runner"] = runner
    return runner


def _device_args(in_maps):
    fn, in_names, out_names, out_avals, mesh, _chain = _get_runner()
    args = []
    for n in in_names:
        if n == "hs":
            args.append(np.concatenate([m["hs"] for m in in_maps], axis=0))
        else:
            args.append(in_maps[0][n])
    for av in out_avals:
        args.append(np.zeros((NCORES * av.shape[0],) + av.shape[1:], av.dtype))
    return args


def kernel(**inputs) -> np.ndarray:
    fn, in_names, out_names, out_avals, mesh, _chain = _get_runner()
    in_maps = _prep_inputs(**inputs)
    args = _device_args(in_maps)
    outs = fn(*args)
    out = np.asarray(outs[0])  # (8*4096, 2048)
    return out.reshape(NCORES, B, TC, U, O).transpose(1, 0, 2, 3, 4).reshape(B, T, U, O)


if __name__ == "__main__":
    import time
    t0 = time.time()
    nc = build_nc(steps=int(sys.argv[1]) if len(sys.argv) > 1 else U)
    print(f"built ok in {time.time()-t0:.1f}s", flush=True)
